# revision 22
# baseline (speedup 1.0000x reference)
"""MoE gate (router) kernel for Trainium2, 8 NeuronCores, data-parallel.

reference: logits = x @ W_g  ([16384,2048] @ [2048,64]); scores = softmax(logits);
           return top-6 (indices, scores).

Strategy (v3: fp16 stream, foldless)
------------------------------------
Data-parallel over tokens: each of the 8 cores handles 2048 tokens. The
contraction dim K=2048 lives on SBUF partitions (host pre-arranges). The
kernel is HBM-bandwidth bound; x and W ship as plain fp16 (2B/elem).

fp16 rounding of x AND W gives ~3.6e-4 max relative error on scores and
~97/16384 rows swap two adjacent, numerically-equal scores (min 6th/7th
score gap on this input is 1.7e-6). Measured off-line: rv_idx ~ 8.8e-4,
rv_val ~ 6e-8 - both far under the 2e-2 gate, vs ~30us of DMA saved
over the 3-byte hi+lo scheme.

v2 carried a Wl correction plane and folded psA[:,:64] + 2^-11*
psA[:,64:] per block; the fold's scalar->DVE->scalar ping-pong put
~450ns of dead time on the in-order scalar queue per hop, making the
epilogue cadence (~1.4us/block) exceed the DMA cadence (1.23us/block) -
the PE then trailed data arrival by ~6us. Dropping Wl (71 -> 97 bad
rows, both noise vs the gate) removes the fold entirely.

Per 128-token block:
  - one PSUM accumulation group of 16 fp16 matmuls into psA [128,64]
  - erow = exp(psA) with fused accumulate reads PSUM directly and gives
    the softmax denominator in one ACT op (no max subtraction needed,
    |logits| < ~6); this is also the only PSUM reader, freeing the bank
  - top-8 value/index via DVE max/max_index on erow (exp is monotone so
    indices match the logits'); max_index writes the [.,8] index staging
  - rec = 1/sum via reciprocal_approx_fast (custom DVE op, ~18 bits,
    faster than table-based InstReciprocal and less DVE-table traffic)
  - scores = v8[:, :6] * rec on DVE (tensor_scalar_mul, per-partition
    scalar)

Engine budget per block (measured op costs): DMA 1.23us (the roofline),
PE 1.07us (16 fp16 matmuls), DVE ~0.88us, scalar ~0.63us.

DMA design (measured on this part):
  - Each DMA splits into one descriptor per partition; each of the 16
    queue engines moves a flat ~26.7 GB/s for descriptors >= 4KiB, so
    the practical ceiling is ~425 GB/s/core. One whole-block DMA (now
    exactly 4KiB/partition) per 128-token block rides it.
  - W rides the scalar ring, in parallel with the x stream on the sync
    ring. Its packets land after x0-x2 (the DGE favors the deep sync
    backlog) so block 0's matmuls start ~1.5us late - but the pipeline
    has 0.18us/block of PE slack, so the lag fully erodes and the END
    time is set by total bytes, not by W's position. Leading the sync
    ring with W (v3) just delayed every x block by W's 1us transfer.
  - Outputs: val (f32, bitcast) and idx8 (u32) share ONE staging tile
    [P, nt, 14] so a block's scores+indices are one contiguous span.
    They leave on the otherwise-idle GPSIMD ring as an early DMA
    (blocks 0..13, overlapped with the x stream) plus a tiny late DMA
    (blocks 14-15), shrinking the post-compute tail. Two tiles (A: 14
    blocks, B: 2 blocks) keep the early DMA's dependencies exact.
"""

import os
import sys

import numpy as np

for _p in ("/opt/trn_rl_repo", "/root/.axon_site/_ro/trn_rl_repo"):
    if os.path.isdir(_p) and _p not in sys.path:
        sys.path.insert(0, _p)

import concourse.bass as bass
import concourse.mybir as mybir
from concourse import bacc, bass_utils
from concourse.tile import TileContext

N_CORES = 8
T_FULL = 16384
K = 2048
E = 64
TOPK = 6
P = 128
KC = K // P  # 16 contraction chunks

_NC_CACHE: dict[int, "bass.Bass"] = {}
LAST_RESULT = None  # BassKernelResults of the most recent kernel() call


def build_nc(t_shard: int = T_FULL // N_CORES) -> "bass.Bass":
    f16 = mybir.dt.float16
    f32 = mybir.dt.float32
    i32 = mybir.dt.int32
    u32 = mybir.dt.uint32
    EXP = mybir.ActivationFunctionType.Exp

    assert t_shard % P == 0
    nt = t_shard // P  # number of 128-token blocks
    SB = KC * P  # 2048 fp16 = 4KiB per (partition, block)

    nt_a = nt - 2  # blocks in the early output DMA

    WE = KC * E  # 1024 f16 of W per partition, leading the x stream
    nc = bacc.Bacc()
    # single input: per partition W (chunk-major [c, E]) then the nt
    # 4KiB token blocks. W + blocks 0-1 ride ONE leading DMA: a DMA's
    # first descriptor batch pays ~0.3-1us of fetch latency whenever the
    # queue has no backlog, which only ever hits the first transfers -
    # merging them leaves zero cold handoffs (blocks 2+ prefetch behind
    # the stream's backlog).
    x_d = nc.dram_tensor("xp", [P, WE + nt * SB], f16, kind="ExternalInput")
    # single merged output, partition-major; host splits/reorders. Per
    # (partition, block): 6 u32-bitcast f32 scores then the full top-8
    # indices (a [:, :6] strided source would shatter into 24-byte
    # descriptors); host keeps the first 6 indices.
    out_o = nc.dram_tensor("out", [P, nt * 14], u32, kind="ExternalOutput")

    with TileContext(nc) as tc:
        with (
            tc.tile_pool(name="singles", bufs=1) as singles,
            tc.tile_pool(name="xpool", bufs=6) as xpool,
            tc.tile_pool(name="small", bufs=6) as small,
            tc.tile_pool(name="psum", bufs=8, space="PSUM") as psum_pool,
        ):
            # leading DMA: W + blocks 0-1 in one 12KiB/partition
            # transfer (see x_d comment). Blocks 0-1's matmuls wait for
            # the whole leading DMA (~+1.2us initial lag vs split DMAs)
            # but the pipeline's 0.18us/block PE slack erodes that well
            # before block 15, so the END time only sees the saved
            # handoff stalls.
            wx01 = singles.tile([P, WE + 2 * SB], f16)
            nc.sync.dma_start(out=wx01, in_=x_d[:, : WE + 2 * SB])
            # merged output staging (see module docstring): per block 6
            # bitcast-f32 scores + 8 u32 indices, split into an early
            # tile (blocks 0..nt_a-1) and a late tile (last 2 blocks) so
            # the early GPSIMD DMA's dependencies are exact.
            ost_a = singles.tile([P, nt_a, 14], u32)
            ost_b = singles.tile([P, nt - nt_a, 14], u32)

            for b in range(nt):
                ost = ost_a[:, b] if b < nt_a else ost_b[:, b - nt_a]
                if b < 2:
                    xb = wx01[:, WE + b * SB : WE + (b + 1) * SB]
                else:
                    xbt = xpool.tile([P, SB], f16, tag="xb")
                    # block DMAs on the sync ring with 4KiB descriptors:
                    # the DMA queues run at a flat ~26.7GB/s each, and a
                    # single ring of whole-block DMAs keeps every queue
                    # saturated.
                    nc.sync.dma_start(
                        out=xbt, in_=x_d[:, WE + b * SB : WE + (b + 1) * SB]
                    )
                    xb = xbt
                psA = psum_pool.tile([P, E], f32, tag="psA")
                # single accumulation group of 16 fp16 matmuls (N=64)
                for c in range(KC):
                    nc.tensor.matmul(
                        psA,
                        xb[:, c * P : (c + 1) * P],
                        wx01[:, c * E : (c + 1) * E],
                        start=(c == 0),
                        stop=(c == KC - 1),
                    )
                # softmax + top-6 (no max subtraction; |logits| < ~6).
                # EXP reads PSUM directly (sole reader -> frees the bank)
                # and the fused accumulate gives the denominator free.
                # max/max_index run on erow = exp(logits): exp is
                # monotone so the indices match the logits'.
                erow = small.tile([P, E], f32, tag="erow")
                sume = small.tile([P, 1], f32, tag="sume")
                nc.scalar.activation(erow, psA, EXP, accum_out=sume)
                v8 = small.tile([P, 8], f32, tag="v8")
                nc.vector.max(out=v8, in_=erow)
                nc.vector.max_index(out=ost[:, 6:14], in_max=v8, in_values=erow)
                rec = small.tile([P, 1], f32, tag="rec")
                nc.vector.reciprocal_approx_fast(out=rec, in_=sume)
                nc.vector.tensor_scalar_mul(
                    ost[:, 0:6].bitcast(f32), v8[:, :TOPK], rec
                )
                if b == nt_a - 1:
                    # early output DMA: blocks 0..nt_a-1 leave on the
                    # idle GPSIMD ring while the x stream still runs.
                    nc.gpsimd.dma_start(
                        out=out_o[:, : nt_a * 14],
                        in_=ost_a[:].rearrange("p b e -> p (b e)"),
                    )

            nc.gpsimd.dma_start(
                out=out_o[:, nt_a * 14 :],
                in_=ost_b[:].rearrange("p b e -> p (b e)"),
            )
    if not nc.is_finalized():
        nc.finalize()
    return nc


def _get_nc(t_shard: int) -> "bass.Bass":
    if t_shard not in _NC_CACHE:
        _NC_CACHE[t_shard] = build_nc(t_shard)
    return _NC_CACHE[t_shard]


def pack_core_input(xh: np.ndarray, Whp: np.ndarray) -> np.ndarray:
    """[t_shard, K] fp16 -> [P, KC*E + nt*2048]: per partition W
    (chunk-major) then per block the 16 chunk rows of 128 tokens, 4KiB
    contiguous."""
    t_shard = xh.shape[0]
    nt = t_shard // P
    out = np.empty((P, KC * E + nt * KC * P), np.float16)
    out[:, : KC * E] = Whp
    hiT = xh.T.reshape(KC, P, nt, P).transpose(1, 2, 0, 3)  # [p, b, c, t]
    out[:, KC * E :] = hiT.reshape(P, nt * KC * P)
    return np.ascontiguousarray(out)


def kernel(x: np.ndarray, W_g: np.ndarray, **run_kwargs):
    global LAST_RESULT
    x = np.asarray(x, dtype=np.float32)
    W = np.asarray(W_g, dtype=np.float32)
    t_shard = x.shape[0] // N_CORES
    nc = _get_nc(t_shard)

    xh = x.astype(np.float16)
    Wh = W.astype(np.float16)  # [K, E]
    Whp = np.ascontiguousarray(
        Wh.reshape(KC, P, E).transpose(1, 0, 2).reshape(P, KC * E)
    )
    in_maps = [
        {"xp": pack_core_input(xh[c * t_shard : (c + 1) * t_shard], Whp)}
        for c in range(N_CORES)
    ]
    res = bass_utils.run_bass_kernel_spmd(
        nc, in_maps, core_ids=list(range(N_CORES)), **run_kwargs
    )
    LAST_RESULT = res
    # device layout is [P, nt, 14] u32 (6 bitcast-f32 scores + top-8
    # idx); token t = tile*P + p -> [t_shard, .]
    nt = t_shard // P
    idx_parts, val_parts = [], []
    for r in res.results:
        o = np.moveaxis(r["out"].reshape(P, nt, 14), 0, 1).reshape(t_shard, 14)
        val_parts.append(o[:, 0:TOPK].copy().view(np.float32))
        idx_parts.append(o[:, 6 : 6 + TOPK].astype(np.int32))
    idx = np.concatenate(idx_parts, axis=0).astype(np.int32)
    val = np.concatenate(val_parts, axis=0).astype(np.float32)
    return idx, val


# revision 23
# speedup vs baseline: 1.0522x; 1.0522x over previous
"""MoE gate (router) kernel for Trainium2, 8 NeuronCores, data-parallel.

reference: logits = x @ W_g  ([16384,2048] @ [2048,64]); scores = softmax(logits);
           return top-6 (indices, scores).

Strategy (v3: fp16 stream, foldless)
------------------------------------
Data-parallel over tokens: each of the 8 cores handles 2048 tokens. The
contraction dim K=2048 lives on SBUF partitions (host pre-arranges). The
kernel is HBM-bandwidth bound; x and W ship as plain fp16 (2B/elem).

fp16 rounding of x AND W gives ~3.6e-4 max relative error on scores and
~97/16384 rows swap two adjacent, numerically-equal scores (min 6th/7th
score gap on this input is 1.7e-6). Measured off-line: rv_idx ~ 8.8e-4,
rv_val ~ 6e-8 - both far under the 2e-2 gate, vs ~30us of DMA saved
over the 3-byte hi+lo scheme.

v2 carried a Wl correction plane and folded psA[:,:64] + 2^-11*
psA[:,64:] per block; the fold's scalar->DVE->scalar ping-pong put
~450ns of dead time on the in-order scalar queue per hop, making the
epilogue cadence (~1.4us/block) exceed the DMA cadence (1.23us/block) -
the PE then trailed data arrival by ~6us. Dropping Wl (71 -> 97 bad
rows, both noise vs the gate) removes the fold entirely.

Per 128-token block:
  - one PSUM accumulation group of 16 fp16 matmuls into psA [128,64]
  - erow = exp(psA) with fused accumulate reads PSUM directly and gives
    the softmax denominator in one ACT op (no max subtraction needed,
    |logits| < ~6); this is also the only PSUM reader, freeing the bank
  - top-8 value/index via DVE max/max_index on erow (exp is monotone so
    indices match the logits'); max_index writes the [.,8] index staging
  - rec = 1/sum via reciprocal_approx_fast (custom DVE op, ~18 bits,
    faster than table-based InstReciprocal and less DVE-table traffic)
  - scores = v8[:, :6] * rec on DVE (tensor_scalar_mul, per-partition
    scalar)

Engine budget per block (measured op costs): DMA 1.23us (the roofline),
PE 1.07us (16 fp16 matmuls), DVE ~0.88us, scalar ~0.63us.

DMA design (measured on this part):
  - Each DMA splits into one descriptor per partition; each of the 16
    queue engines moves a flat ~26.7 GB/s for descriptors >= 4KiB, so
    the practical ceiling is ~425 GB/s/core. One whole-block DMA (now
    exactly 4KiB/partition) per 128-token block rides it.
  - W rides the scalar ring, in parallel with the x stream on the sync
    ring. Its packets land after x0-x2 (the DGE favors the deep sync
    backlog) so block 0's matmuls start ~1.5us late - but the pipeline
    has 0.18us/block of PE slack, so the lag fully erodes and the END
    time is set by total bytes, not by W's position. Leading the sync
    ring with W (v3) just delayed every x block by W's 1us transfer.
  - Outputs: val (f32, bitcast) and idx8 (u32) share ONE staging tile
    [P, nt, 14] so a block's scores+indices are one contiguous span.
    They leave on the otherwise-idle GPSIMD ring as an early DMA
    (blocks 0..13, overlapped with the x stream) plus a tiny late DMA
    (blocks 14-15), shrinking the post-compute tail. Two tiles (A: 14
    blocks, B: 2 blocks) keep the early DMA's dependencies exact.
"""

import os
import sys

import numpy as np

for _p in ("/opt/trn_rl_repo", "/root/.axon_site/_ro/trn_rl_repo"):
    if os.path.isdir(_p) and _p not in sys.path:
        sys.path.insert(0, _p)

import concourse.bass as bass
import concourse.mybir as mybir
from concourse import bacc, bass_utils
from concourse.tile import TileContext

N_CORES = 8
T_FULL = 16384
K = 2048
E = 64
TOPK = 6
P = 128
KC = K // P  # 16 contraction chunks

_NC_CACHE: dict[int, "bass.Bass"] = {}
LAST_RESULT = None  # BassKernelResults of the most recent kernel() call


def build_nc(t_shard: int = T_FULL // N_CORES) -> "bass.Bass":
    f16 = mybir.dt.float16
    f32 = mybir.dt.float32
    i32 = mybir.dt.int32
    u32 = mybir.dt.uint32
    EXP = mybir.ActivationFunctionType.Exp

    assert t_shard % P == 0
    nt = t_shard // P  # number of 128-token blocks
    SB = KC * P  # 2048 fp16 = 4KiB per (partition, block)

    nt_a = nt - 2  # blocks in the early output DMA

    WE = KC * E  # 1024 f16 of W per partition, leading the x stream
    nc = bacc.Bacc()
    # single input: per partition W (chunk-major [c, E]) then the nt
    # 4KiB token blocks. W + blocks 0-1 ride ONE leading DMA: a DMA's
    # first descriptor batch pays ~0.3-1us of fetch latency whenever the
    # queue has no backlog, which only ever hits the first transfers -
    # merging them leaves zero cold handoffs (blocks 2+ prefetch behind
    # the stream's backlog).
    x_d = nc.dram_tensor("xp", [P, WE + nt * SB], f16, kind="ExternalInput")
    # single merged output, partition-major; host splits/reorders. Per
    # (partition, block): 6 u32-bitcast f32 scores then the full top-8
    # indices (a [:, :6] strided source would shatter into 24-byte
    # descriptors); host keeps the first 6 indices.
    out_o = nc.dram_tensor("out", [P, nt * 14], u32, kind="ExternalOutput")

    with TileContext(nc) as tc:
        with (
            tc.tile_pool(name="singles", bufs=1) as singles,
            tc.tile_pool(name="xpool", bufs=6) as xpool,
            tc.tile_pool(name="small", bufs=6) as small,
            tc.tile_pool(name="psum", bufs=8, space="PSUM") as psum_pool,
        ):
            # leading DMA: W + blocks 0-1 in one 12KiB/partition
            # transfer (see x_d comment). Blocks 0-1's matmuls wait for
            # the whole leading DMA (~+1.2us initial lag vs split DMAs)
            # but the pipeline's 0.18us/block PE slack erodes that well
            # before block 15, so the END time only sees the saved
            # handoff stalls.
            wx01 = singles.tile([P, WE + 2 * SB], f16)
            nc.sync.dma_start(out=wx01, in_=x_d[:, : WE + 2 * SB])
            # merged output staging (see module docstring): per block 6
            # bitcast-f32 scores + 8 u32 indices, split into an early
            # tile (blocks 0..nt_a-1) and a late tile (last 2 blocks) so
            # the early GPSIMD DMA's dependencies are exact.
            ost_a = singles.tile([P, nt_a, 14], u32)
            ost_b = singles.tile([P, nt - nt_a, 14], u32)

            for b in range(nt):
                ost = ost_a[:, b] if b < nt_a else ost_b[:, b - nt_a]
                if b < 2:
                    xb = wx01[:, WE + b * SB : WE + (b + 1) * SB]
                else:
                    xbt = xpool.tile([P, SB], f16, tag="xb")
                    # block DMAs on the sync ring with 4KiB descriptors:
                    # the DMA queues run at a flat ~26.7GB/s each, and a
                    # single ring of whole-block DMAs keeps every queue
                    # saturated.
                    nc.sync.dma_start(
                        out=xbt, in_=x_d[:, WE + b * SB : WE + (b + 1) * SB]
                    )
                    xb = xbt
                psA = psum_pool.tile([P, E], f32, tag="psA")
                # single accumulation group of 16 fp16 matmuls (N=64)
                for c in range(KC):
                    nc.tensor.matmul(
                        psA,
                        xb[:, c * P : (c + 1) * P],
                        wx01[:, c * E : (c + 1) * E],
                        start=(c == 0),
                        stop=(c == KC - 1),
                    )
                # softmax + top-6 (no max subtraction; |logits| < ~6).
                # EXP reads PSUM directly (sole reader -> frees the bank)
                # and the fused accumulate gives the denominator free.
                # max/max_index run on erow = exp(logits): exp is
                # monotone so the indices match the logits'.
                erow = small.tile([P, E], f32, tag="erow")
                sume = small.tile([P, 1], f32, tag="sume")
                nc.scalar.activation(erow, psA, EXP, accum_out=sume)
                v8 = small.tile([P, 8], f32, tag="v8")
                nc.vector.max(out=v8, in_=erow)
                nc.vector.max_index(out=ost[:, 6:14], in_max=v8, in_values=erow)
                rec = small.tile([P, 1], f32, tag="rec")
                nc.vector.reciprocal_approx_fast(out=rec, in_=sume)
                nc.vector.tensor_scalar_mul(
                    ost[:, 0:6].bitcast(f32), v8[:, :TOPK], rec
                )
                if b == nt_a - 1:
                    # early output DMA: blocks 0..nt_a-1 leave while the
                    # stream tail still runs. Both output DMAs ride the
                    # SYNC ring: it is idle after its last x trigger,
                    # the triggers sit after every x trigger in program
                    # order (so they cannot gate the input stream), and
                    # a warm ring's first-packet latency is ~1us vs
                    # ~2.1us for a cold (gpsimd) ring.
                    nc.sync.dma_start(
                        out=out_o[:, : nt_a * 14],
                        in_=ost_a[:].rearrange("p b e -> p (b e)"),
                    )

            nc.sync.dma_start(
                out=out_o[:, nt_a * 14 :],
                in_=ost_b[:].rearrange("p b e -> p (b e)"),
            )
    if not nc.is_finalized():
        nc.finalize()
    return nc


def _get_nc(t_shard: int) -> "bass.Bass":
    if t_shard not in _NC_CACHE:
        _NC_CACHE[t_shard] = build_nc(t_shard)
    return _NC_CACHE[t_shard]


def pack_core_input(xh: np.ndarray, Whp: np.ndarray) -> np.ndarray:
    """[t_shard, K] fp16 -> [P, KC*E + nt*2048]: per partition W
    (chunk-major) then per block the 16 chunk rows of 128 tokens, 4KiB
    contiguous."""
    t_shard = xh.shape[0]
    nt = t_shard // P
    out = np.empty((P, KC * E + nt * KC * P), np.float16)
    out[:, : KC * E] = Whp
    hiT = xh.T.reshape(KC, P, nt, P).transpose(1, 2, 0, 3)  # [p, b, c, t]
    out[:, KC * E :] = hiT.reshape(P, nt * KC * P)
    return np.ascontiguousarray(out)


def kernel(x: np.ndarray, W_g: np.ndarray, **run_kwargs):
    global LAST_RESULT
    x = np.asarray(x, dtype=np.float32)
    W = np.asarray(W_g, dtype=np.float32)
    t_shard = x.shape[0] // N_CORES
    nc = _get_nc(t_shard)

    xh = x.astype(np.float16)
    Wh = W.astype(np.float16)  # [K, E]
    Whp = np.ascontiguousarray(
        Wh.reshape(KC, P, E).transpose(1, 0, 2).reshape(P, KC * E)
    )
    in_maps = [
        {"xp": pack_core_input(xh[c * t_shard : (c + 1) * t_shard], Whp)}
        for c in range(N_CORES)
    ]
    res = bass_utils.run_bass_kernel_spmd(
        nc, in_maps, core_ids=list(range(N_CORES)), **run_kwargs
    )
    LAST_RESULT = res
    # device layout is [P, nt, 14] u32 (6 bitcast-f32 scores + top-8
    # idx); token t = tile*P + p -> [t_shard, .]
    nt = t_shard // P
    idx_parts, val_parts = [], []
    for r in res.results:
        o = np.moveaxis(r["out"].reshape(P, nt, 14), 0, 1).reshape(t_shard, 14)
        val_parts.append(o[:, 0:TOPK].copy().view(np.float32))
        idx_parts.append(o[:, 6 : 6 + TOPK].astype(np.int32))
    idx = np.concatenate(idx_parts, axis=0).astype(np.int32)
    val = np.concatenate(val_parts, axis=0).astype(np.float32)
    return idx, val


# revision 24
# speedup vs baseline: 1.1504x; 1.0933x over previous
"""MoE gate (router) kernel for Trainium2, 8 NeuronCores, data-parallel.

reference: logits = x @ W_g  ([16384,2048] @ [2048,64]); scores = softmax(logits);
           return top-6 (indices, scores).

Strategy (fp16 stream, foldless pipeline)
-----------------------------------------
Data-parallel over tokens: each of the 8 cores handles 2048 tokens. The
contraction dim K=2048 lives on SBUF partitions (host pre-arranges). The
kernel is HBM-bandwidth bound; x and W ship as plain fp16 (2B/elem).

fp16 rounding of x AND W gives ~3.6e-4 max relative error on scores and
~97/16384 rows swap two adjacent, numerically-equal scores (min 6th/7th
score gap on this input is 1.7e-6). Measured off-line: rv_idx ~ 8.8e-4,
rv_val ~ 6e-8 - both far under the 2e-2 gate, vs ~10us of DMA saved
over the previous 3-byte fp16+fp8 hi/lo scheme. An intermediate variant
carried a Wl correction plane (71 vs 97 swapped rows) folded as
psA[:,:64] + 2^-11*psA[:,64:], but the fold's scalar->DVE->scalar
ping-pong put ~450ns of dead time per hop on the in-order scalar queue,
pushing the epilogue cadence (~1.4us/block) past the DMA cadence
(1.23us/block); the PE then trailed data arrival by ~6us. Foldless, the
per-block engine budget (measured) is: DMA 1.23us (the roofline), PE
1.07us (16 fp16 matmuls), DVE ~0.88us, scalar ~0.63us - every engine
tracks the stream, and each block's matmuls start ~350ns after its
bytes land.

Per 128-token block:
  - one PSUM accumulation group of 16 fp16 matmuls into psA [128,64]
  - erow = exp(psA) with fused accumulate reads PSUM directly and gives
    the softmax denominator in one ACT op (no max subtraction needed,
    |logits| < ~6); this is also the only PSUM reader, freeing the bank
  - top-8 value/index via DVE max/max_index on erow (exp is monotone so
    indices match the logits'); max_index writes the [.,8] index staging
  - rec = 1/sum via reciprocal_approx_fast (custom DVE op, ~18 bits,
    faster than table-based InstReciprocal and less DVE-table traffic)
  - scores = v8[:, :6] * rec on DVE (tensor_scalar_mul, per-partition
    scalar)

DMA design (all measured on this part; the stream is the roofline):
  - Each DMA splits into one descriptor per partition; each of the 16
    queue engines moves a flat ~26.7 GB/s for descriptors >= 4KiB, so
    the practical ceiling is ~425 GB/s/core. One whole-block DMA
    (exactly 4KiB/partition) per 128-token block rides the sync ring;
    in steady state blocks land every 1.25us.
  - W + blocks 0-1 merge into ONE leading 12KiB/partition DMA: a DMA's
    first descriptor batch pays ~0.3-1us of fetch latency whenever the
    queue lacks backlog, which only ever hits the first transfers.
    Blocks 0-1's matmuls wait on the whole merged DMA, but the pipeline
    erodes the extra initial lag long before block 15.
  - Trigger issue is paced by xpool bufs=6: all-upfront issue (bufs=16)
    left the early stream at ~1.4us/block from descriptor-generation
    contention; paced, it locks to the flat 1.25us.
  - Outputs: val (f32, bitcast) and idx8 (u32) share ONE staging tile
    layout [P, nt, 14] so a block's scores+indices are one contiguous
    span (idx ships the full top-8; a [:, :6] strided source shatters
    into 24-byte descriptors). They leave as an early DMA (blocks
    0..13, fired right after block 13's scores while the stream tail
    still runs) plus a tiny late DMA (blocks 14-15). Both ride the
    WARM sync ring (~1us first-packet latency vs ~2.1us cold), safely
    after every x trigger in program order. Two tiles (A/B) keep the
    early DMA's dependencies exact.

Known fixed costs (not removable kernel-side): ~6.5us framework
preamble (engine barriers, DVE table register loads), ~2.1us DGE
trigger-to-first-packet latency, ~2.5us teardown barrier, and periodic
~1us DVE-table refresh DMAs on queue engine E64 every ~8-9us that
straggle one block each when they land mid-stream.
"""

import os
import sys

import numpy as np

for _p in ("/opt/trn_rl_repo", "/root/.axon_site/_ro/trn_rl_repo"):
    if os.path.isdir(_p) and _p not in sys.path:
        sys.path.insert(0, _p)

import concourse.bass as bass
import concourse.mybir as mybir
from concourse import bacc, bass_utils
from concourse.tile import TileContext

N_CORES = 8
T_FULL = 16384
K = 2048
E = 64
TOPK = 6
P = 128
KC = K // P  # 16 contraction chunks

_NC_CACHE: dict[int, "bass.Bass"] = {}
LAST_RESULT = None  # BassKernelResults of the most recent kernel() call


def build_nc(t_shard: int = T_FULL // N_CORES) -> "bass.Bass":
    f16 = mybir.dt.float16
    f32 = mybir.dt.float32
    u32 = mybir.dt.uint32
    EXP = mybir.ActivationFunctionType.Exp

    assert t_shard % P == 0
    nt = t_shard // P  # number of 128-token blocks
    SB = KC * P  # 2048 fp16 = 4KiB per (partition, block)

    nt_a = nt - 2  # blocks in the early output DMA

    WE = KC * E  # 1024 f16 of W per partition, leading the x stream
    nc = bacc.Bacc()
    # single input: per partition W (chunk-major [c, E]) then the nt
    # 4KiB token blocks. W + blocks 0-1 ride ONE leading DMA: a DMA's
    # first descriptor batch pays ~0.3-1us of fetch latency whenever the
    # queue has no backlog, which only ever hits the first transfers -
    # merging them leaves zero cold handoffs (blocks 2+ prefetch behind
    # the stream's backlog).
    x_d = nc.dram_tensor("xp", [P, WE + nt * SB], f16, kind="ExternalInput")
    # single merged output, partition-major; host splits/reorders. Per
    # (partition, block): 6 u32-bitcast f32 scores then the full top-8
    # indices (a [:, :6] strided source would shatter into 24-byte
    # descriptors); host keeps the first 6 indices.
    out_o = nc.dram_tensor("out", [P, nt * 14], u32, kind="ExternalOutput")

    with TileContext(nc) as tc:
        with (
            tc.tile_pool(name="singles", bufs=1) as singles,
            tc.tile_pool(name="xpool", bufs=6) as xpool,
            tc.tile_pool(name="small", bufs=6) as small,
            tc.tile_pool(name="psum", bufs=8, space="PSUM") as psum_pool,
        ):
            # leading DMA: W + blocks 0-1 in one 12KiB/partition
            # transfer (see x_d comment). Blocks 0-1's matmuls wait for
            # the whole leading DMA (~+1.2us initial lag vs split DMAs)
            # but the pipeline's 0.18us/block PE slack erodes that well
            # before block 15, so the END time only sees the saved
            # handoff stalls.
            wx01 = singles.tile([P, WE + 2 * SB], f16)
            nc.sync.dma_start(out=wx01, in_=x_d[:, : WE + 2 * SB])
            # merged output staging (see module docstring): per block 6
            # bitcast-f32 scores + 8 u32 indices, split into an early
            # tile (blocks 0..nt_a-1) and a late tile (last 2 blocks) so
            # the early GPSIMD DMA's dependencies are exact.
            ost_a = singles.tile([P, nt_a, 14], u32)
            ost_b = singles.tile([P, nt - nt_a, 14], u32)

            for b in range(nt):
                ost = ost_a[:, b] if b < nt_a else ost_b[:, b - nt_a]
                if b < 2:
                    xb = wx01[:, WE + b * SB : WE + (b + 1) * SB]
                else:
                    xbt = xpool.tile([P, SB], f16, tag="xb")
                    # block DMAs on the sync ring with 4KiB descriptors:
                    # the DMA queues run at a flat ~26.7GB/s each, and a
                    # single ring of whole-block DMAs keeps every queue
                    # saturated.
                    nc.sync.dma_start(
                        out=xbt, in_=x_d[:, WE + b * SB : WE + (b + 1) * SB]
                    )
                    xb = xbt
                psA = psum_pool.tile([P, E], f32, tag="psA")
                # single accumulation group of 16 fp16 matmuls (N=64)
                for c in range(KC):
                    nc.tensor.matmul(
                        psA,
                        xb[:, c * P : (c + 1) * P],
                        wx01[:, c * E : (c + 1) * E],
                        start=(c == 0),
                        stop=(c == KC - 1),
                    )
                # softmax + top-6 (no max subtraction; |logits| < ~6).
                # EXP reads PSUM directly (sole reader -> frees the bank)
                # and the fused accumulate gives the denominator free.
                # max/max_index run on erow = exp(logits): exp is
                # monotone so the indices match the logits'.
                erow = small.tile([P, E], f32, tag="erow")
                sume = small.tile([P, 1], f32, tag="sume")
                nc.scalar.activation(erow, psA, EXP, accum_out=sume)
                v8 = small.tile([P, 8], f32, tag="v8")
                nc.vector.max(out=v8, in_=erow)
                nc.vector.max_index(out=ost[:, 6:14], in_max=v8, in_values=erow)
                rec = small.tile([P, 1], f32, tag="rec")
                nc.vector.reciprocal_approx_fast(out=rec, in_=sume)
                nc.vector.tensor_scalar_mul(
                    ost[:, 0:6].bitcast(f32), v8[:, :TOPK], rec
                )
                if b == nt_a - 1:
                    # early output DMA: blocks 0..nt_a-1 leave while the
                    # stream tail still runs. Both output DMAs ride the
                    # SYNC ring: it is idle after its last x trigger,
                    # the triggers sit after every x trigger in program
                    # order (so they cannot gate the input stream), and
                    # a warm ring's first-packet latency is ~1us vs
                    # ~2.1us for a cold (gpsimd) ring.
                    nc.sync.dma_start(
                        out=out_o[:, : nt_a * 14],
                        in_=ost_a[:].rearrange("p b e -> p (b e)"),
                    )

            nc.sync.dma_start(
                out=out_o[:, nt_a * 14 :],
                in_=ost_b[:].rearrange("p b e -> p (b e)"),
            )
    if not nc.is_finalized():
        nc.finalize()
    return nc


def _get_nc(t_shard: int) -> "bass.Bass":
    if t_shard not in _NC_CACHE:
        _NC_CACHE[t_shard] = build_nc(t_shard)
    return _NC_CACHE[t_shard]


def pack_core_input(xh: np.ndarray, Whp: np.ndarray) -> np.ndarray:
    """[t_shard, K] fp16 -> [P, KC*E + nt*2048]: per partition W
    (chunk-major) then per block the 16 chunk rows of 128 tokens, 4KiB
    contiguous."""
    t_shard = xh.shape[0]
    nt = t_shard // P
    out = np.empty((P, KC * E + nt * KC * P), np.float16)
    out[:, : KC * E] = Whp
    hiT = xh.T.reshape(KC, P, nt, P).transpose(1, 2, 0, 3)  # [p, b, c, t]
    out[:, KC * E :] = hiT.reshape(P, nt * KC * P)
    return np.ascontiguousarray(out)


def kernel(x: np.ndarray, W_g: np.ndarray, **run_kwargs):
    global LAST_RESULT
    x = np.asarray(x, dtype=np.float32)
    W = np.asarray(W_g, dtype=np.float32)
    t_shard = x.shape[0] // N_CORES
    nc = _get_nc(t_shard)

    xh = x.astype(np.float16)
    Wh = W.astype(np.float16)  # [K, E]
    Whp = np.ascontiguousarray(
        Wh.reshape(KC, P, E).transpose(1, 0, 2).reshape(P, KC * E)
    )
    in_maps = [
        {"xp": pack_core_input(xh[c * t_shard : (c + 1) * t_shard], Whp)}
        for c in range(N_CORES)
    ]
    res = bass_utils.run_bass_kernel_spmd(
        nc, in_maps, core_ids=list(range(N_CORES)), **run_kwargs
    )
    LAST_RESULT = res
    # device layout is [P, nt, 14] u32 (6 bitcast-f32 scores + top-8
    # idx); token t = tile*P + p -> [t_shard, .]
    nt = t_shard // P
    idx_parts, val_parts = [], []
    for r in res.results:
        o = np.moveaxis(r["out"].reshape(P, nt, 14), 0, 1).reshape(t_shard, 14)
        val_parts.append(o[:, 0:TOPK].copy().view(np.float32))
        idx_parts.append(o[:, 6 : 6 + TOPK].astype(np.int32))
    idx = np.concatenate(idx_parts, axis=0).astype(np.int32)
    val = np.concatenate(val_parts, axis=0).astype(np.float32)
    return idx, val
